# revision 27
# baseline (speedup 1.0000x reference)
"""Multi-head attention (B=4, S=1024, D=1024, H=16, DH=64) on 8 trn2 cores.

Tensor-parallel over heads: core c owns heads {2c, 2c+1}; each core runs
8 independent attention units (4 batches x 2 heads).  Per-head projections
only read a 64-channel slice of the input, so each core receives just its
2x64-channel slice, pre-transposed to [d, s] with a ones-row appended
(E1 = 66: row 64 is the ones row, 65 zero pad).

Math per unit (b, h).  The Wk^T.Wq product is folded on the host
(G^T = Wq~ @ Wk~^T, biases/scale included via the ones-row), so only one
projection feeds the scores:
  y[c,s]    = G^T.T @ xTe           (one 66x66 "projection" replaces q,k)
  scT[t,s]  = xTe.T @ y             (= q.k scores, transposed: t on parts)
  v[t,e']   = xTe.T @ WvTe2         (col 64 == 1 -> Z column of out2)
  expT      = exp(scT) -> bf16      (no max-subtraction: |scores| <= ~10)
  out2[s,e']= sum_t expT[t,s] v[t,e']   (transposed PV: s on partitions,
                                     col 64 = Z[s]; per s-block 8
                                     accumulating 66-row bf16 matmuls)
  out[s,e]  = out2[s,e] / Z[s]      (per-partition reciprocal multiply)

The kernel is ACT(exp)-paced: exp of the full scores is ~55us of
unavoidable Activation-engine element time plus a fixed ~185ns init per
activation instruction.  To minimize the instruction count the scores
stream through rotating 3-bank PSUM "generation" tiles ([128, 1536]
f32, bufs=2: 6 banks) written in 512-col pieces and consumed by exp in
1536-col chunks (44 chunks instead of 64), each chunk spanning t-block
boundaries but never a generation boundary, so each tile sees only ~3
writers and <=2 readers and the tile framework's range-based dependency
tracking stays fine-grained.  Everything else (projections, the
previous unit's transposed-PV + normalize + store) is interleaved into
the piece/exp stream as filler chunks on a shared 2-bank PSUM pool,
keeping PE/DVE/SP below the ACT roofline.  A dummy 2-element exp right
at kernel start pulls the 1.3us activation-table load into the DMA fill
window.  One batched DMA per unit stores [128, 512] with 2KB contiguous
lines (the last unit's store is split into quarters to shorten the
drain).  Engine busy at 73.3us total: ACT 64us (gap-free between fill
and tail), PE 50us, DVE 40us, SP 19us.
"""

import numpy as np

D = 1024
H = 16
DH = 64
B = 4
S = 1024
NCORES = 8
HPC = H // NCORES  # heads per core = 2
E1 = DH + 2  # 66: ones-row at 64, zero pad at 65
NT = S // 128  # 8 t/s blocks
NU = B * HPC  # 8 units per core
SCALE = 1.0 / np.sqrt(DH)

GEN = 1536  # scores buffer generation: one 3-bank psum tile, in columns
# exp chunk sizes: a short prefix so ACT starts early, then 1536-col
# chunks; chunks never straddle a generation boundary.
CHUNKS = [512, 1024] + [1536] * 41 + [1024]
assert sum(CHUNKS) == NU * NT * S  # 65536 scores columns per core
CSTART = np.concatenate([[0], np.cumsum(CHUNKS)])
assert all(
    CSTART[i] // GEN == (CSTART[i] + CHUNKS[i] - 1) // GEN
    for i in range(len(CHUNKS))
)

_CACHE = {}


def _chunk_of(g):
    """Chunk index and in-chunk offset for global scores column g."""
    c = int(np.searchsorted(CSTART, g, side="right")) - 1
    return c, g - int(CSTART[c])


def _split_sync_waits(nc, limit=1):
    """Walrus in this toolchain rejects instructions carrying more than one
    sync-wait; peel extra waits onto wait-only EventSemaphore ops inserted
    just before, on the same engine queue (engine streams are in-order)."""
    import concourse.mybir as mybir

    n = 0
    for bb in nc.main_func.blocks:
        out = []
        for ins in bb.instructions:
            si = ins.sync_info
            if si is not None and len(si.on_wait) > limit:
                waits = list(si.on_wait)
                for w in waits[:-limit]:
                    ev = mybir.InstEventSemaphore(
                        name=f"WSPLIT-{n}", ins=[], outs=[]
                    )
                    n += 1
                    ev.engine = ins.engine
                    ev.sync_info = mybir.SyncInfo(on_wait=[w], on_update=[])
                    out.append(ev)
                ins.sync_info = mybir.SyncInfo(
                    on_wait=waits[-limit:], on_update=list(si.on_update)
                )
            out.append(ins)
        bb.instructions = out
    return n


def _build_bass(split=True):
    import concourse.bass as bass
    import concourse.mybir as mybir
    import concourse.tile as tile

    f32 = mybir.dt.float32
    f32r = mybir.dt.float32r
    bf16 = mybir.dt.bfloat16
    nc = bass.Bass()

    xTe_d = nc.declare_dram_parameter("xTe", [B, HPC, E1, S], f32r, isOutput=False)
    gt_d = nc.declare_dram_parameter("GT", [E1, HPC * E1], f32r, isOutput=False)
    wv_d = nc.declare_dram_parameter("WvTe2", [E1, HPC * E1], f32r, isOutput=False)
    # out[b, j, p, blk*64 + e] == attention(b, s=blk*128+p, head j)[e]
    out_d = nc.declare_dram_parameter("out", [B, HPC, 128, 512], f32, isOutput=True)

    with tile.TileContext(nc) as tc:
        with (
            tc.tile_pool(name="const", bufs=1) as constp,
            tc.tile_pool(name="sb", bufs=2) as sbp,
            tc.tile_pool(name="expp", bufs=2) as expp,
            tc.tile_pool(name="psR", bufs=1, space="PSUM") as psR,
            tc.tile_pool(name="psM", bufs=2, space="PSUM") as psM,
        ):
            # dummy activation: pulls the exp table load into the DMA fill
            dummy = constp.tile([1, 4], f32, name="dummy")
            nc.gpsimd.memset(dummy[:], 0.0)
            nc.scalar.activation(
                dummy[:, 2:4], dummy[:, 0:2], mybir.ActivationFunctionType.Exp
            )

            gt_sb = constp.tile([E1, HPC * E1], f32r)
            wv_sb = constp.tile([E1, HPC * E1], f32r)
            nc.gpsimd.dma_start(gt_sb[:], gt_d[:])
            nc.gpsimd.dma_start(wv_sb[:], wv_d[:])

            gens = {}  # generation idx -> rotating 3-bank scores tile

            def new_gen(i):
                gens[i] = psR.tile([128, GEN], f32, tag="sc", bufs=2,
                                   name=f"sc_{i}")

            units = [(b, j) for b in range(B) for j in range(HPC)]

            xts = {}

            def fetch_xt(b, j):
                if (b, j) not in xts:
                    for jj in range(HPC):
                        xts[(b, jj)] = sbp.tile(
                            [E1, S], f32r, tag="xt", bufs=4,
                            name=f"xt_{b}_{jj}",
                        )
                    for jj in range(HPC):  # j-major: head 0 complete first
                        for half in range(2):
                            hs = slice(half * 512, (half + 1) * 512)
                            nc.sync.dma_start(
                                xts[(b, jj)][:, hs], xTe_d[b, jj, :, hs]
                            )
                return xts[(b, j)]

            def y_chunks(b, j):
                """Scores projection y = G^T.T @ x as 2 filler chunks."""
                xt = fetch_xt(b, j)
                yT = sbp.tile([E1, S], f32r, tag="yT", bufs=3, name=f"y_{b}_{j}")
                chunks = []
                for sh in range(2):
                    def chunk(sh=sh):
                        ss = slice(sh * 512, (sh + 1) * 512)
                        y_ps = psM.tile(
                            [128, 512], f32, tag="m", bufs=2, name="y_ps"
                        )
                        nc.tensor.matmul(
                            y_ps[:E1, :],
                            gt_sb[:, j * E1:(j + 1) * E1],
                            xt[:, ss],
                            start=True, stop=True,
                        )
                        nc.vector.tensor_copy(yT[:, ss], y_ps[:E1, :])
                    chunks.append(chunk)
                return yT, chunks

            def v_chunks(b, j):
                """v projection as 2 filler chunks (4 MMs + bf16 copy each).

                v_sb[t, tb*E1 + e] is cast to bf16 by the psum->sbuf copy
                so the transposed-PV matmuls (bf16 x bf16) run 1 cyc/row;
                the projection itself stays f32r (the real compiler rejects
                mixed 32/16-bit matmul inputs).
                """
                xt = fetch_xt(b, j)
                v_sb = sbp.tile(
                    [128, NT * E1], bf16, tag="v", bufs=4, name=f"v_{b}_{j}"
                )
                chunks = []
                for half in range(2):
                    def chunk(half=half):
                        v_ps = psM.tile(
                            [128, 512], f32, tag="m", bufs=2, name="v_ps"
                        )
                        for q in range(4):
                            tb = half * 4 + q
                            nc.tensor.matmul(
                                v_ps[:, q * E1:(q + 1) * E1],
                                xt[:, tb * 128:(tb + 1) * 128],
                                wv_sb[:, j * E1:(j + 1) * E1],
                                start=True, stop=True,
                            )
                        nc.vector.tensor_copy(
                            v_sb[:, half * 4 * E1:(half + 1) * 4 * E1],
                            v_ps[:, :4 * E1],
                        )
                    chunks.append(chunk)
                return v_sb, chunks

            chunk_exp = {}  # chunk idx -> expT tile

            def pv_chunks(u, v_sb):
                """Deferred transposed-PV + normalize + store for unit u.

                One chunk per s-block: 8 accumulating 66-row matmuls (one
                psum group, alone in its bank), then 1/Z normalize into the
                unit's output tile; a final chunk DMAs the unit out."""
                b, j = units[u]
                o_sb = sbp.tile([128, 512], f32, tag="o", bufs=2,
                                name=f"o_{b}_{j}")
                chunks = []
                for sb in range(NT):
                    def chunk(sb=sb):
                        out2 = psM.tile([128, 512], f32, tag="m", bufs=2,
                                        name=f"pv_{b}_{j}_{sb}")
                        for tb in range(NT):
                            c, off = _chunk_of(u * 8192 + tb * 1024 + sb * 128)
                            nc.tensor.matmul(
                                out2[:, :E1],
                                chunk_exp[c][:, off:off + 128],
                                v_sb[:, tb * E1:(tb + 1) * E1],
                                start=(tb == 0), stop=(tb == NT - 1),
                            )
                        invz = sbp.tile(
                            [128, 1], f32, tag="invz", bufs=8, name="invz"
                        )
                        nc.vector.reciprocal(invz[:], out2[:, DH:DH + 1])
                        nc.vector.tensor_scalar_mul(
                            o_sb[:, sb * DH:(sb + 1) * DH],
                            out2[:, :DH],
                            invz[:],
                        )
                    chunks.append(chunk)

                if u == NU - 1:  # split the last store to shorten drain
                    def dma_q(q):
                        def go():
                            nc.sync.dma_start(
                                out_d[b, j][:, q * 128:(q + 1) * 128],
                                o_sb[:, q * 128:(q + 1) * 128],
                            )
                        return go
                    for q in reversed(range(4)):  # after each sb pair
                        chunks.insert(2 * q + 2, dma_q(q))
                else:
                    def dma_chunk():
                        nc.sync.dma_start(out_d[b, j], o_sb[:])
                    chunks.append(dma_chunk)
                return chunks

            def last_pv_chunks(u, v_sb):
                """Last unit's PV split so almost nothing trails the final
                exp.  Per s-block, the accumulating matmuls are emitted in
                waves keyed by which exp chunk their expT slice lives in
                (cA = third-last, cB = second-last, cF = final), parked in
                psum banks already done with scores (the two psM banks plus
                free regions of the two scores tiles).  After the final exp
                only one matmul per s-block remains, then the normalize
                (spread over DVE divide / Pool divide / ACT copy-scale) and
                two half stores."""
                b, j = units[u]
                o_sb = sbp.tile([128, 512], f32, tag="o", bufs=2,
                                name=f"o_{b}_{j}")
                cF = c_end[u]
                gF = int(CSTART[cF]) // GEN   # gen tile of the final chunk
                fre = (int(CSTART[cF]) % GEN + CHUNKS[cF]) // 512  # used regions
                regions = {}

                def gen_tile(i):
                    for k in range(max(gens) + 1, i + 1):
                        new_gen(k)  # keep pool-rotation allocation order
                    return gens[i]

                def region(sb):
                    if sb not in regions:
                        if sb < 2:
                            t = psM.tile([128, 512], f32, tag="m", bufs=2,
                                         name=f"lpv_{sb}")
                            regions[sb] = t[:, :E1]
                        elif sb < 5:
                            off = (sb - 2) * 512
                            regions[sb] = gen_tile(gF - 1)[:, off:off + E1]
                        else:
                            off = ((sb - 5 + fre) % 3) * 512
                            regions[sb] = gen_tile(gF)[:, off:off + E1]
                    return regions[sb]

                def tb_chunk(sb, tb):
                    return _chunk_of(u * 8192 + tb * 1024 + sb * 128)

                started = [False] * NT

                def mms(sb, tbs):
                    out2 = region(sb)
                    for tb in tbs:
                        c, off = tb_chunk(sb, tb)
                        nc.tensor.matmul(
                            out2,
                            chunk_exp[c][:, off:off + 128],
                            v_sb[:, tb * E1:(tb + 1) * E1],
                            start=(tb == tbs[0] and not started[sb]),
                            stop=(tb == NT - 1),
                        )
                    started[sb] = True

                # s-blocks whose psum region overlaps data an exp still
                # has to read may not open their group before that exp is
                # emitted -- and the scores tiles' dep tracking is coarse,
                # so anything in the final tile (sb5-7) effectively waits
                # the final exp and is left entirely to the fin phase.
                SAFE_A = (0, 1)
                SAFE_B = (0, 1, 2, 3, 4)

                def wave(ceil_c, allowed):
                    """Chunks emitting, per s-block, the accumulation matmuls
                    whose expT chunk index is <= ceil_c (and > the previous
                    wave's ceiling, tracked via `done`)."""
                    chs = []
                    for sb in allowed:
                        tbs = [tb for tb in range(NT)
                               if done[sb] <= tb and tb_chunk(sb, tb)[0] <= ceil_c]
                        if tbs:
                            def ch(sb=sb, tbs=tbs):
                                mms(sb, tbs)
                            chs.append(ch)
                            done[sb] = tbs[-1] + 1
                    return chs

                done = [0] * NT
                preA = wave(cF - 2, SAFE_A)
                preB = wave(cF - 1, SAFE_B)

                # normalize engines: GPSIMD cannot touch PSUM and DVE has
                # no divide on real hw, so alternate DVE multiply with an
                # ACT copy whose per-partition scale is 1/Z (native
                # activation scale operand); the reciprocals stay on DVE.
                NORM_ENG = ["dve", "dve", "dve", "dve",
                            "dve", "dve", "dve", "dve"]
                fin = []
                for sb in range(NT):
                    def fin_chunk(sb=sb):
                        out2 = region(sb)
                        mms(sb, list(range(done[sb], NT)))
                        o_sl = o_sb[:, sb * DH:(sb + 1) * DH]
                        invz = sbp.tile(
                            [128, 1], f32, tag="invz", bufs=8, name="invz"
                        )
                        nc.vector.reciprocal(invz[:], out2[:, DH:DH + 1])
                        if NORM_ENG[sb] == "act":
                            nc.scalar.activation(
                                o_sl, out2[:, :DH],
                                mybir.ActivationFunctionType.Copy,
                                scale=invz[:],
                            )
                        else:
                            nc.vector.tensor_scalar_mul(
                                o_sl, out2[:, :DH], invz[:]
                            )
                    fin.append(fin_chunk)

                def dma_h(h):
                    def go():
                        nc.sync.dma_start(
                            out_d[b, j][:, h * 256:(h + 1) * 256],
                            o_sb[:, h * 256:(h + 1) * 256],
                        )
                    return go
                fin.insert(4, dma_h(0))
                fin.append(dma_h(1))
                return preA, preB, fin

            # Software pipeline: scores pieces stream through the psum ring
            # in 512-col steps; exp fires per chunk; projection chunks of
            # the next unit and PV/normalize/store chunks of the previous
            # unit interleave as fillers.
            from collections import deque

            fillers = deque()
            unit_io = {}

            def unit_inputs(u):
                b, j = units[u]
                yT, ychunks = y_chunks(b, j)
                for c in ychunks:
                    fillers.append(c)
                v_sb, vchunks = v_chunks(b, j)
                for c in vchunks:
                    fillers.append(c)
                unit_io[u] = (yT, v_sb)

            # chunk index whose exp completes each unit's scores
            c_end = [_chunk_of((u + 1) * 8192 - 128)[0] for u in range(NU)]

            unit_inputs(0)
            # unit 0 fill: alternate projection chunks with scores pieces so
            # each piece's inputs are the only thing it waits on (a chunk
            # blocked on a later DMA would stall earlier-ready pieces).
            u0_chunks = list(fillers)
            fillers.clear()
            for p in range(NU * 16):  # 512-col scores pieces
                g = p * 512
                u, tb, sh = g // 8192, (g % 8192) // 1024, (g % 1024) // 512
                if p < len(u0_chunks):
                    u0_chunks[p]()  # y sh0 before p0, y sh1 before p1, v...
                if p % 16 == 4 and u + 1 < NU:
                    unit_inputs(u + 1)
                yT, _ = unit_io[u]
                if g % GEN == 0 and g // GEN not in gens:
                    new_gen(g // GEN)
                rp = g % GEN
                nc.tensor.matmul(
                    gens[g // GEN][:, rp:rp + 512],
                    fetch_xt(*units[u])[:, tb * 128:(tb + 1) * 128],
                    yT[:, sh * 512:(sh + 1) * 512],
                    start=True, stop=True,
                )
                c, _ = _chunk_of(g)
                if g + 512 == int(CSTART[c + 1]):  # chunk complete -> exp
                    csz = CHUNKS[c]
                    cp = int(CSTART[c]) % GEN
                    expT = expp.tile(
                        [128, 1536], bf16, tag="expT", bufs=16, name="expT"
                    )
                    nc.scalar.activation(
                        expT[:, :csz],
                        gens[int(CSTART[c]) // GEN][:, cp:cp + csz],
                        mybir.ActivationFunctionType.Exp,
                    )
                    chunk_exp[c] = expT
                    for uu in range(NU):
                        if c_end[uu] == c:
                            for ch in pv_chunks(uu, unit_io[uu][1]):
                                fillers.append(ch)
                for _ in range(2):
                    if fillers:
                        fillers.popleft()()
            while fillers:
                fillers.popleft()()
    if split:
        _split_sync_waits(nc)
    return nc


def _prep_inputs(sequences, Wq, Wk, Wv, bq, bk, bv):
    """Host-side packing: per-core input maps."""
    import ml_dtypes

    sequences = np.ascontiguousarray(np.asarray(sequences, dtype=np.float32))
    Wq = np.asarray(Wq, np.float32)
    Wk = np.asarray(Wk, np.float32)
    Wv = np.asarray(Wv, np.float32)
    bq = np.asarray(bq, np.float32)
    bk = np.asarray(bk, np.float32)
    bv = np.asarray(bv, np.float32)

    # [B, S, H, DH] -> [H, B, DH, S] transposed slices
    xT = np.ascontiguousarray(
        sequences.reshape(B, S, H, DH).transpose(2, 0, 3, 1)
    )  # [H, B, DH, S]

    in_maps = []
    for c in range(NCORES):
        heads = [HPC * c + j for j in range(HPC)]
        xTe = np.zeros((B, HPC, E1, S), np.float32)
        xTe[:, :, DH, :] = 1.0
        for j, h in enumerate(heads):
            xTe[:, j, :DH, :] = xT[h]
        gt = np.zeros((E1, HPC, E1), np.float32)
        wv = np.zeros((E1, HPC, E1), np.float32)
        for j, h in enumerate(heads):
            wq = np.zeros((E1, DH), np.float32)  # x~ -> q, scale folded
            wq[:DH] = Wq[h].T * SCALE
            wq[DH] = bq[h] * SCALE
            wk = np.zeros((E1, DH), np.float32)  # x~ -> k
            wk[:DH] = Wk[h].T
            wk[DH] = bk[h]
            # scores = k.q = x~^T (Wk~ Wq~^T) x~; lhsT of the y-projection
            # is the transpose: G^T = Wq~ @ Wk~^T
            gt[:, j, :] = wq @ wk.T
            wv[:DH, j, :DH] = Wv[h].T
            wv[DH, j, :DH] = bv[h]
            wv[DH, j, DH] = 1.0  # ones column -> Z column of out2
        in_maps.append({
            "xTe": xTe,
            "GT": gt.reshape(E1, HPC * E1),
            "WvTe2": wv.reshape(E1, HPC * E1),
        })
    return in_maps


def get_nc():
    if "nc" not in _CACHE:
        _CACHE["nc"] = _build_bass()
    return _CACHE["nc"]


def kernel(sequences, Wq, Wk, Wv, bq, bk, bv):
    from concourse.bass_utils import run_bass_kernel_spmd

    nc = get_nc()
    in_maps = _prep_inputs(sequences, Wq, Wk, Wv, bq, bk, bv)
    res = run_bass_kernel_spmd(nc, in_maps, list(range(NCORES)))
    full = np.empty((B, S, D), np.float32)
    for c in range(NCORES):
        # out[b, j, p, blk*64+e] -> full[b, blk*128+p, (2c+j)*64+e]
        arr = res.results[c]["out"].reshape(B, HPC, 128, NT, DH)
        full[:, :, c * HPC * DH:(c + 1) * HPC * DH] = (
            arr.transpose(0, 3, 2, 1, 4).reshape(B, S, HPC * DH)
        )
    return full
